# revision 18
# baseline (speedup 1.0000x reference)
"""Trainium2 Bass kernel for nn_BakedAttentionHead — fp8 DoubleRow version.

Reference computation (per row b of query):
    s      = (q @ K^T) / sqrt(D)                      # (B, N)
    e'     = exp(s - max_n s)
    d      = 1 + sum_n e'
    recip  = 16-step sigmoid long-division approx of 1/d
    out    = (e' * recip) @ V

Kernel restructuring (matches reference to ~6e-3 of absmax, gate is 2e-2):
    Every matmul runs as a 3-term fp8(e4m3) DoubleRow decomposition:
    X ~= X8 + Xr8 (value + rounded residual, split on host for Q/K/V and
    on-chip for the exp'd weights), and  A@B ~= A8@B8 + Ar8@B8 + A8@Br8,
    dropping only the O(0.06%) Ar8@Br8 term.  DoubleRow packs two
    128-contraction slots per PE pass at 0.5 cycles/row, so the 3x term
    count nets 0.75x of the fp32r matmul time.

    e_scaled = exp(s + ln(1/4))        (raw exp pre-scaled to fit e4m3's
                                        240 max; descale folds into em)
    em       = 4 * exp(-max_n s)       (ACT Exp bias = ln 4)
    d        = 1 + (sum_n e_scaled) * em
    out      = (e_scaled @ V) * (em * recip(d)) per row, stored bf16

    Row sums come free from a DoubleRow ones-matmul over the (e8, er8)
    tiles (partition reduce); only row-max stats go through PE transposes.

Engine balance per mm1 tile ([128,512] psum, 1.28us PE pace):
    ACT  e32 = Exp(psum)  then  e8 = Copy(e32)->fp8      (~1.22us)
    Pool er8 = e32 - e8 -> fp8                           (~1.11us)
    DVE  macc = max(psum, macc)                          (~0.66us)
The last nt tile's chain is latency-split (halves, er8 on idle DVE) so
mm2's first group, whose nt15-dependent passes are emitted last, never
waits.  The reciprocal scan's sigmoids are emitted interleaved with the
mm2 groups (strict per-engine FIFOs!), after a dummy sigmoid that
preloads the ACT table set behind em.

Sharding: data-parallel over the 8192 query rows -> 8 cores x 1024 rows,
keys/values replicated.  mm1 computes scores^T ([n, m]) so the exp'd fp8
tiles are directly the lhsT operand of mm2.  All DMA slices keep >=512B
contiguous runs (smaller descriptors cost 2x).
"""

import numpy as np
import ml_dtypes

F8NP = ml_dtypes.float8_e4m3
BF16NP = ml_dtypes.bfloat16

B, D, N = 8192, 1024, 2048
NCORES = 8
M = B // NCORES            # 1024 query rows per core
NWIN = 2                   # m windows per core (one mm1 sweep each)
PW = M // NWIN             # 512 m per window
MT = PW // 128             # 4 m-chunks of 128 rows per window
HT = PW // 256             # 2 half-width (256 m) psum groups per window
NT = N // 128              # 16 n tiles
DT = D // 128              # 8 d (contraction) tiles
SCALE = 0.03125            # D ** -0.5
LN_WSCALE = float(np.log(0.25))   # weight pre-scale bias: e4m3 max is 240
LN_DESCALE = float(np.log(4.0))   # descale folded into em
SIG_SCALE = 100.0
BITS = 16
N_EARLY = 5                # mm2 groups evacuated unscaled via ACT copy

_CACHE = {}


def _build(reps=1):
    import concourse.mybir as mybir
    import concourse.tile as tile
    from concourse import bacc
    from concourse.masks import make_identity
    from concourse.tile import add_dep_helper

    F32 = mybir.dt.float32
    F8 = mybir.dt.float8e4
    BF16 = mybir.dt.bfloat16
    AX = mybir.AxisListType
    OP = mybir.AluOpType
    AF = mybir.ActivationFunctionType
    PM = mybir.MatmulPerfMode.DoubleRow

    nc = bacc.Bacc("TRN2", target_bir_lowering=False, debug=False,
                   num_devices=NCORES)
    q8_d = nc.declare_dram_parameter("q8", [D, M], F8, isOutput=False)
    qr8_d = nc.declare_dram_parameter("qr8", [D, M], F8, isOutput=False)
    k8_d = nc.declare_dram_parameter("k8", [D, N], F8, isOutput=False)
    kr8_d = nc.declare_dram_parameter("kr8", [D, N], F8, isOutput=False)
    v8_d = nc.declare_dram_parameter("v8", [N, D], F8, isOutput=False)
    vr8_d = nc.declare_dram_parameter("vr8", [N, D], F8, isOutput=False)
    out_d = nc.declare_dram_parameter("out", [M, D], BF16, isOutput=True)

    q8_ap = q8_d[:].rearrange("(dt p) m -> p dt m", p=128)
    qr8_ap = qr8_d[:].rearrange("(dt p) m -> p dt m", p=128)
    k8_ap = k8_d[:].rearrange("(dt p) n -> p dt n", p=128)
    kr8_ap = kr8_d[:].rearrange("(dt p) n -> p dt n", p=128)
    v8_ap = v8_d[:].rearrange("(nt p) d -> p nt d", p=128)
    vr8_ap = vr8_d[:].rearrange("(nt p) d -> p nt d", p=128)

    with tile.TileContext(nc) as tc:
        with (
            tc.tile_pool(name="res", bufs=1) as res_pool,
            tc.tile_pool(name="e", bufs=2) as e_pool,
            tc.tile_pool(name="e32", bufs=3) as e32_pool,
            tc.tile_pool(name="acc", bufs=2) as acc_pool,
            tc.tile_pool(name="stat", bufs=2) as stat_pool,
            tc.tile_pool(name="o", bufs=8) as out_pool,
            tc.tile_pool(name="ps1", bufs=3, space="PSUM") as ps1_pool,
            tc.tile_pool(name="ps2", bufs=3, space="PSUM") as ps2_pool,
            tc.tile_pool(name="pst", bufs=1, space="PSUM") as pst_pool,
            tc.tile_pool(name="pss", bufs=1, space="PSUM") as pss_pool,
        ):
            ident = res_pool.tile([128, 128], F32)
            make_identity(nc, ident[:])
            bias_w = res_pool.tile([128, 1], F32)
            nc.vector.memset(bias_w[:], LN_WSCALE)
            bias_d = res_pool.tile([128, 1], F32)
            nc.vector.memset(bias_d[:], LN_DESCALE)
            ones32 = res_pool.tile([128, 1], F32)
            nc.vector.memset(ones32[:], 1.0)
            sigdump = res_pool.tile([128, 1], F32)

            for rep in range(reps):
                # SBUF staging: kind-interleaved fp8 tensors so both the
                # main-term APs (fixed kind, dt/nt pair as the DoubleRow
                # slot dim) and the cross-term APs (kind as the slot dim)
                # are regular slices.
                #   kcat kinds: [0]=k8  [1]=kr8      qcat: [0]=qr8 [1]=q8
                #   vcat kinds: [0]=vr8 [1]=v8       ecat: [0]=e8  [1]=er8
                # cross mm1: (k8, kr8) x (qr8, q8) -> K8'Qr8 + Kr8'Q8
                # cross mm2: (e8, er8) x (vr8, v8) -> E8'Vr8 + Er8'V8
                kcat = res_pool.tile([128, 2, DT, N], F8)
                qcat = res_pool.tile([128, 2, DT, M], F8)
                vcat = res_pool.tile([128, 2, NT, D], F8)

                # FIFO DMA order = first-use order.  k8[0:512]+q8w0 unblock
                # nt0-2 main terms; kr8/qr8 unblock the cross terms ~3us
                # later (mm1's first nts emit main-only to cover the gap).
                hd = DT // 2
                nc.sync.dma_start(out=kcat[:, 0, 0:hd, 0:512],
                                  in_=k8_ap[:, 0:hd, 0:512])
                nc.sync.dma_start(out=qcat[:, 1, 0:hd, 0:PW],
                                  in_=q8_ap[:, 0:hd, 0:PW])
                nc.sync.dma_start(out=kcat[:, 0, hd:DT, 0:512],
                                  in_=k8_ap[:, hd:DT, 0:512])
                nc.sync.dma_start(out=qcat[:, 1, hd:DT, 0:PW],
                                  in_=q8_ap[:, hd:DT, 0:PW])
                nc.sync.dma_start(out=kcat[:, 1, 0:hd, 0:512],
                                  in_=kr8_ap[:, 0:hd, 0:512])
                nc.sync.dma_start(out=qcat[:, 0, 0:hd, 0:PW],
                                  in_=qr8_ap[:, 0:hd, 0:PW])
                nc.sync.dma_start(out=kcat[:, 1, hd:DT, 0:512],
                                  in_=kr8_ap[:, hd:DT, 0:512])
                nc.sync.dma_start(out=qcat[:, 0, hd:DT, 0:PW],
                                  in_=qr8_ap[:, hd:DT, 0:PW])
                for c in range(1, 4):
                    n0, n1 = c * 512, (c + 1) * 512
                    nc.sync.dma_start(out=kcat[:, 0, :, n0:n1],
                                      in_=k8_ap[:, :, n0:n1])
                    nc.sync.dma_start(out=kcat[:, 1, :, n0:n1],
                                      in_=kr8_ap[:, :, n0:n1])
                # V by d-halves: mm2(w0) does all dc=0 groups first.
                for dc in range(2):
                    d0, d1 = dc * 512, (dc + 1) * 512
                    nc.sync.dma_start(out=vcat[:, 1, :, d0:d1],
                                      in_=v8_ap[:, :, d0:d1])
                    nc.sync.dma_start(out=vcat[:, 0, :, d0:d1],
                                      in_=vr8_ap[:, :, d0:d1])
                nc.sync.dma_start(out=qcat[:, 1, :, PW:M], in_=q8_ap[:, :, PW:M])
                nc.sync.dma_start(out=qcat[:, 0, :, PW:M],
                                  in_=qr8_ap[:, :, PW:M])

                def mm1_tile(ps, nt, m0):
                    # One complete accumulation group per 256-m half: a
                    # [128,512] f32 psum tile is exactly one 2KB zero
                    # region, so a later start would mark the sibling
                    # half's bytes pending-zero and the next accumulate
                    # into them silently overwrites.
                    ns = slice(nt * 128, (nt + 1) * 128)
                    for h in range(HT):
                        hs = slice(h * 256, (h + 1) * 256)
                        qs = slice(m0 + h * 256, m0 + (h + 1) * 256)
                        for t in range(DT // 2):
                            nc.tensor.matmul(
                                ps[:, hs],
                                lhsT=kcat[:, 0, 2 * t:2 * t + 2, ns],
                                rhs=qcat[:, 1, 2 * t:2 * t + 2, qs],
                                start=(t == 0), stop=False, perf_mode=PM)
                        for dt in range(DT):
                            nc.tensor.matmul(
                                ps[:, hs],
                                lhsT=kcat[:, :, dt, ns],
                                rhs=qcat[:, :, dt, qs],
                                start=False, stop=(dt == DT - 1), perf_mode=PM)

                def mm1_evac(w, ecat, macc, ps, nt):
                    """e32 (ACT) -> e8 (ACT even nt / DVE odd nt, so the
                    ACT queue keeps pace even with the previous window's
                    mm2 psum-evac copies in it) -> er8 (Pool); macc over the
                    e32 tiles on DVE (max of e_scaled: its reciprocal IS em,
                    so no ACT exp sits on the late stats path).  Last tile:
                    e8 halves on ACT, er8 halves deferred to the stats
                    weave."""
                    e32 = e32_pool.tile([128, PW], F32,
                                        name=f"e32_{w}_{nt}", tag="e32")
                    nc.scalar.activation(e32[:], ps[:], AF.Exp,
                                         bias=bias_w[:], scale=SCALE)
                    if nt < NT - 1:
                        if nt % 2 == 0:
                            nc.scalar.activation(ecat[:, 0, nt, :], e32[:],
                                                 AF.Copy)
                        else:
                            nc.vector.tensor_copy(ecat[:, 0, nt, :], e32[:])
                        nc.gpsimd.tensor_tensor(out=ecat[:, 1, nt, :],
                                                in0=e32[:],
                                                in1=ecat[:, 0, nt, :],
                                                op=OP.subtract)
                        if nt == 0:
                            nc.vector.tensor_copy(macc[:], e32[:])
                        else:
                            nc.vector.tensor_tensor(
                                out=macc[:], in0=e32[:], in1=macc[:],
                                op=OP.max)
                    else:
                        # macc first: it gates the stats transposes and must
                        # not queue behind anything else in the DVE FIFO
                        nc.vector.tensor_tensor(
                            out=macc[:], in0=e32[:], in1=macc[:], op=OP.max)
                        for h in range(HT):
                            hs = slice(h * 256, (h + 1) * 256)
                            nc.scalar.activation(ecat[:, 0, nt, hs], e32[:, hs],
                                                 AF.Copy)
                        # dummy sigmoid: preloads the ACT sigmoid table set
                        # at the earliest possible queue slot, so the scan's
                        # first real sigmoid doesn't pay the 1.28us load
                        dummy = nc.scalar.activation(sigdump[:], bias_w[:],
                                                     AF.Sigmoid)
                        return e32, dummy
                    return e32, None

                def emit_late_er8(w, ecat, e32, h):
                    hs = slice(h * 256, (h + 1) * 256)
                    nc.vector.tensor_tensor(out=ecat[:, 1, NT - 1, hs],
                                            in0=e32[:, hs],
                                            in1=ecat[:, 0, NT - 1, hs],
                                            op=OP.subtract)

                def emit_sum_mms(pssum, e32s, nt):
                    # single accumulation group across all nts AND columns:
                    # a per-column start would re-mark the whole 2KB zero
                    # region and zero the sibling columns' partial sums
                    for c in range(MT):
                        nc.tensor.matmul(
                            pssum[:, c:c + 1],
                            lhsT=e32s[nt][:, c * 128:(c + 1) * 128],
                            rhs=ones32[:], start=(nt == 0 and c == 0),
                            stop=(nt == NT - 1 and c == MT - 1),
                            skip_group_check=True)

                def emit_mm1(w, mid_cb=None):
                    m0 = w * PW
                    ecat = e_pool.tile([128, 2, NT, PW], F8, name=f"ecat{w}",
                                       tag="ecat")
                    macc = acc_pool.tile([128, PW], F32, name=f"macc{w}",
                                         tag="macc")
                    pssum = pss_pool.tile([128, MT], F32, name=f"pss{w}",
                                          tag="pss")
                    e32s = {}
                    for nt in range(NT):
                        ps = ps1_pool.tile([128, PW], F32, name=f"s{w}_{nt}",
                                           tag="ps1")
                        mm1_tile(ps, nt, m0)
                        # row sums: tiny f32 ones-matmuls over the e32 tiles,
                        # two tiles behind so the PE never waits on ACT
                        if nt >= 2:
                            emit_sum_mms(pssum, e32s, nt - 2)
                        e32s[nt], dummy = mm1_evac(w, ecat, macc, ps, nt)
                        if nt == NT // 2 - 1 and mid_cb is not None:
                            mid_cb()
                    return ecat, macc, pssum, e32s, dummy

                def emit_transposes(w, macc):
                    # all 4 transposes pipeline through one psum bank: each
                    # writes its own quarter (single-instruction groups), so
                    # no transpose waits on the previous quarter's reduce
                    mx = stat_pool.tile([128, MT], F32, name=f"mx{w}", tag="mx")
                    pt = pst_pool.tile([128, MT, 128], F32, name=f"tm{w}",
                                       tag="pst")
                    for c in range(MT):
                        nc.tensor.transpose(
                            pt[:, c, :], macc[:, c * 128:(c + 1) * 128],
                            ident[:])
                    for c in range(MT):
                        nc.vector.tensor_reduce(
                            mx[:, c:c + 1], pt[:, c, :], axis=AX.X, op=OP.max)
                    return mx

                def emit_stats_d(w, st, mx, sm):
                    # mx is max_n e_scaled, so em = 4*exp(-scale*row_max) is
                    # exactly its reciprocal -- computed on DVE, keeping the
                    # ACT queue free for the sigmoid table prefetch below
                    nc.vector.reciprocal(st["em"][:], mx[:])
                    tmp = stat_pool.tile([128, MT], F32, name=f"dt{w}",
                                         tag="dtmp")
                    nc.vector.tensor_tensor(out=tmp[:], in0=sm[:],
                                            in1=st["em"][:], op=OP.mult)
                    nc.vector.tensor_scalar_add(st["d"][:], tmp[:], 1.0)

                def emit_mm2_group(w, ecat, mc, dc, otiles, unscaled,
                                   part=None, ps=None, copy_after=None):
                    """One [128m, 512d] psum group, 3-term fp8 DoubleRow.
                    nt14/15-dependent passes are emitted last so the group
                    can start while mm1's tail e-split chain drains.  part
                    'A' emits only h0's nt<14 passes (nothing from the tail
                    e-split); part 'B' emits the rest + the evac copy."""
                    m0 = w * PW
                    if ps is None:
                        ps = ps2_pool.tile([128, 512], F32,
                                           name=f"o{w}_{mc}_{dc}", tag="ps2")
                    ms = slice(mc * 128, (mc + 1) * 128)
                    for h in range(2):
                        if part == "A" and h == 1:
                            break
                        hs = slice(h * 256, (h + 1) * 256)
                        ds = slice(dc * 512 + h * 256, dc * 512 + (h + 1) * 256)
                        if not (part == "B" and h == 0):
                            for nt in range(NT - 2):
                                nc.tensor.matmul(
                                    ps[:, hs], lhsT=ecat[:, :, nt, ms],
                                    rhs=vcat[:, :, nt, ds],
                                    start=(nt == 0), stop=False, perf_mode=PM)
                            for t in range(NT // 2 - 1):
                                nc.tensor.matmul(
                                    ps[:, hs],
                                    lhsT=ecat[:, 0, 2 * t:2 * t + 2, ms],
                                    rhs=vcat[:, 1, 2 * t:2 * t + 2, ds],
                                    start=False, stop=False, perf_mode=PM)
                        if part == "A":
                            return ps
                        t = NT // 2 - 1
                        nc.tensor.matmul(
                            ps[:, hs],
                            lhsT=ecat[:, 0, 2 * t:2 * t + 2, ms],
                            rhs=vcat[:, 1, 2 * t:2 * t + 2, ds],
                            start=False, stop=False, perf_mode=PM)
                        for nt in (NT - 2, NT - 1):
                            nc.tensor.matmul(
                                ps[:, hs], lhsT=ecat[:, :, nt, ms],
                                rhs=vcat[:, :, nt, ds],
                                start=False, stop=(nt == NT - 1), perf_mode=PM)
                    if mc not in otiles:
                        otiles[mc] = out_pool.tile([128, D], BF16,
                                                   name=f"ot{w}_{mc}", tag="ot")
                    oth = otiles[mc][:, dc * 512:(dc + 1) * 512]
                    cp = nc.scalar.activation(oth, ps[:], AF.Copy)
                    if copy_after is not None:
                        add_dep_helper(copy_after.ins, cp.ins, True,
                                       "evac copy trails sigmoid-table dummy")
                    unscaled.append((mc, dc))

                def emit_scan_steps(w, st, sstate, steps):
                    """Sigmoid long-division steps on d [128, 4].  The q
                    update of step i-1 is emitted after step i's z so the
                    DVE executes it inside step i's sigmoid round-trip
                    (strict FIFO); sg alternates so it survives a step."""
                    d_t = st["d"]
                    r0, r1, q0, q1, z, sg0, sg1, t = sstate["tiles"]
                    for i in steps:
                        r = sstate["r"]
                        rn = r1 if r is r0 else r0
                        sg = sg0 if i % 2 == 0 else sg1
                        nc.vector.scalar_tensor_tensor(      # z = 2r - d
                            out=z[:], in0=r[:], scalar=2.0, in1=d_t[:],
                            op0=OP.mult, op1=OP.subtract)
                        if sstate["pending_q"] is not None:
                            pi, psg = sstate["pending_q"]
                            qa = sstate["q"]
                            qn = q1 if qa is q0 else q0
                            nc.vector.scalar_tensor_tensor(  # q' = w*step + q
                                out=qn[:], in0=psg[:],
                                scalar=float(2.0 ** -(pi + 1)),
                                in1=qa[:], op0=OP.mult, op1=OP.add)
                            sstate["q"] = qn
                        nc.scalar.activation(                # step = sig(100 z)
                            sg[:], z[:], AF.Sigmoid, scale=SIG_SCALE)
                        nc.vector.tensor_tensor(             # t = d*step
                            out=t[:], in0=d_t[:], in1=sg[:], op=OP.mult)
                        nc.vector.scalar_tensor_tensor(      # r' = 2r - t
                            out=rn[:], in0=r[:], scalar=2.0, in1=t[:],
                            op0=OP.mult, op1=OP.subtract)
                        sstate["r"] = rn
                        sstate["pending_q"] = (i, sg)
                    if steps and steps[-1] == BITS - 1:
                        pi, psg = sstate["pending_q"]
                        qa = sstate["q"]
                        qn = q1 if qa is q0 else q0
                        nc.vector.scalar_tensor_tensor(
                            out=qn[:], in0=psg[:],
                            scalar=float(2.0 ** -(pi + 1)),
                            in1=qa[:], op0=OP.mult, op1=OP.add)
                        sstate["pending_q"] = None
                        nc.vector.tensor_tensor(
                            out=st["scale"][:], in0=st["em"][:],
                            in1=qn[:], op=OP.mult)

                def make_scan_state(w):
                    tiles = [stat_pool.tile([128, MT], F32, name=f"{nm}{w}",
                                            tag=nm)
                             for nm in ("r0", "r1", "q0", "q1", "z", "sg0",
                                        "sg1", "t")]
                    nc.vector.memset(tiles[0][:], 1.0)
                    nc.vector.memset(tiles[2][:], 0.0)
                    return {"tiles": tiles, "r": tiles[0], "q": tiles[2],
                            "pending_q": None}

                def emit_out(w, st, otiles, unscaled):
                    """Scale the evacuated halves (bf16 in-place, cheap),
                    storing each merged [128, 1024] row block with a single
                    DMA (fewer descriptors -> half the HWDGE cost/byte) as
                    soon as both its halves are scaled."""
                    m0 = w * PW
                    for mc in sorted(otiles):
                        for dc in range(2):
                            oth = otiles[mc][:, dc * 512:(dc + 1) * 512]
                            nc.vector.tensor_scalar_mul(
                                oth, oth, st["scale"][:, mc:mc + 1])
                        nc.sync.dma_start(
                            out=out_d[m0 + mc * 128:m0 + (mc + 1) * 128, :],
                            in_=otiles[mc][:])

                pending_out = []

                def flush_out():
                    while pending_out:
                        emit_out(*pending_out.pop(0))

                for w in range(NWIN):
                    st = {
                        "em": stat_pool.tile([128, MT], F32, name=f"em{w}",
                                             tag="em"),
                        "d": stat_pool.tile([128, MT], F32, name=f"d{w}",
                                            tag="d"),
                        "scale": stat_pool.tile([128, MT], F32, name=f"sc{w}",
                                                tag="sc"),
                    }
                    ecat, macc, pssum, e32s, dummy = emit_mm1(
                        w, mid_cb=flush_out)
                    otiles, unscaled = {}, []
                    # boundary weave: keep the PE busy on work that doesn't
                    # need the nt15 e-split while its chain drains, and put
                    # the DVE reduces ahead of the late er8 halves
                    emit_sum_mms(pssum, e32s, NT - 2)
                    ps_g0 = emit_mm2_group(w, ecat, 0, 0, otiles, unscaled,
                                           part="A")
                    emit_late_er8(w, ecat, e32s[NT - 1], 0)
                    emit_late_er8(w, ecat, e32s[NT - 1], 1)
                    mx = emit_transposes(w, macc)
                    emit_sum_mms(pssum, e32s, NT - 1)
                    sm = stat_pool.tile([128, MT], F32, name=f"sm{w}",
                                        tag="sm")
                    nc.vector.tensor_copy(sm[:], pssum[:])
                    emit_stats_d(w, st, mx, sm)
                    sstate = make_scan_state(w)
                    # dc-major group order so mm2(w0) can start before the
                    # V d[512:] half lands.  Interleave scan emission with
                    # the groups: per-engine FIFOs are strict, so each
                    # group's ACT psum-evac copy must sit between sigmoids
                    # matching its psum-ready time (~2.65 sigmoid paces per
                    # group), and the first sigmoids go before g0's copy.
                    groups = [(mc, dc) for dc in range(2) for mc in range(MT)]
                    sched = [(None, 2), (0, 5), (1, 8), (2, 11), (3, 14),
                             (4, BITS), (5, BITS), (6, BITS), (7, BITS)]
                    done = 0
                    for gi, nsig in sched:
                        if gi == 0:
                            emit_mm2_group(w, ecat, *groups[gi], otiles,
                                           unscaled, part="B", ps=ps_g0,
                                           copy_after=dummy)
                        elif gi is not None:
                            emit_mm2_group(w, ecat, *groups[gi], otiles,
                                           unscaled)
                        emit_scan_steps(w, st, sstate,
                                        list(range(done, nsig)))
                        done = nsig
                    pending_out.append((w, st, otiles, unscaled))
                flush_out()

    nc.compile()
    return nc


def _get_nc():
    if "nc" not in _CACHE:
        _CACHE["nc"] = _build()
    return _CACHE["nc"]


def _split8(x):
    h = x.astype(F8NP)
    l = (x - h.astype(np.float32)).astype(F8NP)
    return np.ascontiguousarray(h), np.ascontiguousarray(l)


def prepare_in_maps(query, keys, values):
    query = np.ascontiguousarray(query, dtype=np.float32)
    keys = np.ascontiguousarray(keys, dtype=np.float32)
    values = np.ascontiguousarray(values, dtype=np.float32)
    k8, kr8 = _split8(np.ascontiguousarray(keys.T))
    v8, vr8 = _split8(values)
    in_maps = []
    for i in range(NCORES):
        qT = np.ascontiguousarray(query[i * M:(i + 1) * M].T)
        q8, qr8 = _split8(qT)
        in_maps.append({"q8": q8, "qr8": qr8, "k8": k8, "kr8": kr8,
                        "v8": v8, "vr8": vr8})
    return in_maps


def kernel(query, keys, values):
    from concourse.bass_utils import run_bass_kernel_spmd

    nc = _get_nc()
    in_maps = prepare_in_maps(query, keys, values)
    res = run_bass_kernel_spmd(nc, in_maps, list(range(NCORES)))
    out = np.concatenate([np.asarray(res.results[i]["out"])
                          for i in range(NCORES)], axis=0)
    return out.astype(np.float32)


# revision 19
# speedup vs baseline: 2.6042x; 2.6042x over previous
"""Trainium2 Bass kernel for nn_BakedAttentionHead — fp8 DoubleRow version.

Reference computation (per row b of query):
    s      = (q @ K^T) / sqrt(D)                      # (B, N)
    e'     = exp(s - max_n s)
    d      = 1 + sum_n e'
    recip  = 16-step sigmoid long-division approx of 1/d
    out    = (e' * recip) @ V

Kernel restructuring (matches reference to ~6e-3 of absmax, gate is 2e-2):
    Every matmul runs as a 3-term fp8(e4m3) DoubleRow decomposition:
    X ~= X8 + Xr8 (value + rounded residual, split on host for Q/K/V and
    on-chip for the exp'd weights), and  A@B ~= A8@B8 + Ar8@B8 + A8@Br8,
    dropping only the O(0.06%) Ar8@Br8 term.  DoubleRow packs two
    128-contraction slots per PE pass at 0.5 cycles/row, so the 3x term
    count nets 0.75x of the fp32r matmul time.

    e_scaled = exp(s + ln(1/4))        (raw exp pre-scaled to fit e4m3's
                                        240 max; descale folds into em)
    em       = 4 * exp(-max_n s)       (ACT Exp bias = ln 4)
    d        = 1 + (sum_n e_scaled) * em
    out      = (e_scaled @ V) * (em * recip(d)) per row, stored bf16

    Row sums come free from a DoubleRow ones-matmul over the (e8, er8)
    tiles (partition reduce); only row-max stats go through PE transposes.

Engine balance per mm1 tile ([128,512] psum, 1.28us PE pace):
    ACT  e32 = Exp(psum)  then  e8 = Copy(e32)->fp8      (~1.22us)
    Pool er8 = e32 - e8 -> fp8                           (~1.11us)
    DVE  macc = max(psum, macc)                          (~0.66us)
The last nt tile's chain is latency-split (halves, er8 on idle DVE) so
mm2's first group, whose nt15-dependent passes are emitted last, never
waits.  The reciprocal scan's sigmoids are emitted interleaved with the
mm2 groups (strict per-engine FIFOs!), after a dummy sigmoid that
preloads the ACT table set behind em.

Sharding: data-parallel over the 8192 query rows -> 8 cores x 1024 rows,
keys/values replicated.  mm1 computes scores^T ([n, m]) so the exp'd fp8
tiles are directly the lhsT operand of mm2.  All DMA slices keep >=512B
contiguous runs (smaller descriptors cost 2x).
"""

import numpy as np
import ml_dtypes

F8NP = ml_dtypes.float8_e4m3
BF16NP = ml_dtypes.bfloat16

B, D, N = 8192, 1024, 2048
NCORES = 8
M = B // NCORES            # 1024 query rows per core
NWIN = 2                   # m windows per core (one mm1 sweep each)
PW = M // NWIN             # 512 m per window
MT = PW // 128             # 4 m-chunks of 128 rows per window
HT = PW // 256             # 2 half-width (256 m) psum groups per window
NT = N // 128              # 16 n tiles
DT = D // 128              # 8 d (contraction) tiles
SCALE = 0.03125            # D ** -0.5
LN_WSCALE = float(np.log(0.25))   # weight pre-scale bias: e4m3 max is 240
LN_DESCALE = float(np.log(4.0))   # descale folded into em
SIG_SCALE = 100.0
BITS = 16
N_EARLY = 5                # mm2 groups evacuated unscaled via ACT copy

_CACHE = {}


def _build(reps=1):
    import concourse.mybir as mybir
    import concourse.tile as tile
    from concourse import bacc
    from concourse.masks import make_identity
    from concourse.tile import add_dep_helper

    F32 = mybir.dt.float32
    F8 = mybir.dt.float8e4
    BF16 = mybir.dt.bfloat16
    AX = mybir.AxisListType
    OP = mybir.AluOpType
    AF = mybir.ActivationFunctionType
    PM = mybir.MatmulPerfMode.DoubleRow

    nc = bacc.Bacc("TRN2", target_bir_lowering=False, debug=False,
                   num_devices=NCORES)
    q8_d = nc.declare_dram_parameter("q8", [D, M], F8, isOutput=False)
    qr8_d = nc.declare_dram_parameter("qr8", [D, M], F8, isOutput=False)
    k8_d = nc.declare_dram_parameter("k8", [D, N], F8, isOutput=False)
    kr8_d = nc.declare_dram_parameter("kr8", [D, N], F8, isOutput=False)
    v8_d = nc.declare_dram_parameter("v8", [N, D], F8, isOutput=False)
    vr8_d = nc.declare_dram_parameter("vr8", [N, D], F8, isOutput=False)
    out_d = nc.declare_dram_parameter("out", [M, D], BF16, isOutput=True)

    q8_ap = q8_d[:].rearrange("(dt p) m -> p dt m", p=128)
    qr8_ap = qr8_d[:].rearrange("(dt p) m -> p dt m", p=128)
    k8_ap = k8_d[:].rearrange("(dt p) n -> p dt n", p=128)
    kr8_ap = kr8_d[:].rearrange("(dt p) n -> p dt n", p=128)
    v8_ap = v8_d[:].rearrange("(nt p) d -> p nt d", p=128)
    vr8_ap = vr8_d[:].rearrange("(nt p) d -> p nt d", p=128)

    with tile.TileContext(nc) as tc:
        with (
            tc.tile_pool(name="res", bufs=1) as res_pool,
            tc.tile_pool(name="e", bufs=2) as e_pool,
            tc.tile_pool(name="e32", bufs=3) as e32_pool,
            tc.tile_pool(name="acc", bufs=2) as acc_pool,
            tc.tile_pool(name="stat", bufs=2) as stat_pool,
            tc.tile_pool(name="o", bufs=8) as out_pool,
            tc.tile_pool(name="ps1", bufs=3, space="PSUM") as ps1_pool,
            tc.tile_pool(name="ps2", bufs=3, space="PSUM") as ps2_pool,
            tc.tile_pool(name="pst", bufs=1, space="PSUM") as pst_pool,
            tc.tile_pool(name="pss", bufs=1, space="PSUM") as pss_pool,
        ):
            ident = res_pool.tile([128, 128], F32)
            make_identity(nc, ident[:])
            # PE clock warm-up: a dummy transpose as the very first PE op
            # starts the p-state ramp window (~3us to full clock) before the
            # head DMAs finish, so the first real matmuls run at 2.4GHz
            warm = pst_pool.tile([128, 128], F32, name="warm", tag="pst")
            nc.tensor.transpose(warm[:], ident[:], ident[:])
            bias_w = res_pool.tile([128, 1], F32)
            nc.vector.memset(bias_w[:], LN_WSCALE)
            bias_d = res_pool.tile([128, 1], F32)
            nc.vector.memset(bias_d[:], LN_DESCALE)
            ones32 = res_pool.tile([128, 1], F32)
            nc.vector.memset(ones32[:], 1.0)
            sigdump = res_pool.tile([128, 1], F32)

            for rep in range(reps):
                # SBUF staging: kind-interleaved fp8 tensors so both the
                # main-term APs (fixed kind, dt/nt pair as the DoubleRow
                # slot dim) and the cross-term APs (kind as the slot dim)
                # are regular slices.
                #   kcat kinds: [0]=k8  [1]=kr8      qcat: [0]=qr8 [1]=q8
                #   vcat kinds: [0]=vr8 [1]=v8       ecat: [0]=e8  [1]=er8
                # cross mm1: (k8, kr8) x (qr8, q8) -> K8'Qr8 + Kr8'Q8
                # cross mm2: (e8, er8) x (vr8, v8) -> E8'Vr8 + Er8'V8
                kcat = res_pool.tile([128, 2, DT, N], F8)
                qcat = res_pool.tile([128, 2, DT, M], F8)
                vcat = res_pool.tile([128, 2, NT, D], F8)

                # FIFO DMA order = first-use order.  k8[0:512]+q8w0 unblock
                # nt0-2 main terms; kr8/qr8 unblock the cross terms ~3us
                # later (mm1's first nts emit main-only to cover the gap).
                hd = DT // 2
                nc.sync.dma_start(out=kcat[:, 0, 0:hd, 0:512],
                                  in_=k8_ap[:, 0:hd, 0:512])
                nc.sync.dma_start(out=qcat[:, 1, 0:hd, 0:PW],
                                  in_=q8_ap[:, 0:hd, 0:PW])
                nc.sync.dma_start(out=kcat[:, 0, hd:DT, 0:512],
                                  in_=k8_ap[:, hd:DT, 0:512])
                nc.sync.dma_start(out=qcat[:, 1, hd:DT, 0:PW],
                                  in_=q8_ap[:, hd:DT, 0:PW])
                nc.sync.dma_start(out=kcat[:, 1, 0:hd, 0:512],
                                  in_=kr8_ap[:, 0:hd, 0:512])
                nc.sync.dma_start(out=qcat[:, 0, 0:hd, 0:PW],
                                  in_=qr8_ap[:, 0:hd, 0:PW])
                nc.sync.dma_start(out=kcat[:, 1, hd:DT, 0:512],
                                  in_=kr8_ap[:, hd:DT, 0:512])
                nc.sync.dma_start(out=qcat[:, 0, hd:DT, 0:PW],
                                  in_=qr8_ap[:, hd:DT, 0:PW])
                for c in range(1, 4):
                    n0, n1 = c * 512, (c + 1) * 512
                    nc.sync.dma_start(out=kcat[:, 0, :, n0:n1],
                                      in_=k8_ap[:, :, n0:n1])
                    nc.sync.dma_start(out=kcat[:, 1, :, n0:n1],
                                      in_=kr8_ap[:, :, n0:n1])
                # V by d-halves: mm2(w0) does all dc=0 groups first.
                for dc in range(2):
                    d0, d1 = dc * 512, (dc + 1) * 512
                    nc.sync.dma_start(out=vcat[:, 1, :, d0:d1],
                                      in_=v8_ap[:, :, d0:d1])
                    nc.sync.dma_start(out=vcat[:, 0, :, d0:d1],
                                      in_=vr8_ap[:, :, d0:d1])
                nc.sync.dma_start(out=qcat[:, 1, :, PW:M], in_=q8_ap[:, :, PW:M])
                nc.sync.dma_start(out=qcat[:, 0, :, PW:M],
                                  in_=qr8_ap[:, :, PW:M])

                def mm1_tile(ps, nt, m0):
                    # One complete accumulation group per 256-m half: a
                    # [128,512] f32 psum tile is exactly one 2KB zero
                    # region, so a later start would mark the sibling
                    # half's bytes pending-zero and the next accumulate
                    # into them silently overwrites.
                    ns = slice(nt * 128, (nt + 1) * 128)
                    for h in range(HT):
                        hs = slice(h * 256, (h + 1) * 256)
                        qs = slice(m0 + h * 256, m0 + (h + 1) * 256)
                        for t in range(DT // 2):
                            nc.tensor.matmul(
                                ps[:, hs],
                                lhsT=kcat[:, 0, 2 * t:2 * t + 2, ns],
                                rhs=qcat[:, 1, 2 * t:2 * t + 2, qs],
                                start=(t == 0), stop=False, perf_mode=PM)
                        for dt in range(DT):
                            nc.tensor.matmul(
                                ps[:, hs],
                                lhsT=kcat[:, :, dt, ns],
                                rhs=qcat[:, :, dt, qs],
                                start=False, stop=(dt == DT - 1), perf_mode=PM)

                def mm1_evac(w, ecat, macc, ps, nt):
                    """e32 (ACT) -> e8 (ACT even nt / DVE odd nt, so the
                    ACT queue keeps pace even with the previous window's
                    mm2 psum-evac copies in it) -> er8 (Pool); macc over the
                    e32 tiles on DVE (max of e_scaled: its reciprocal IS em,
                    so no ACT exp sits on the late stats path).  Last tile:
                    e8 halves on ACT, er8 halves deferred to the stats
                    weave."""
                    e32 = e32_pool.tile([128, PW], F32,
                                        name=f"e32_{w}_{nt}", tag="e32")
                    nc.scalar.activation(e32[:], ps[:], AF.Exp,
                                         bias=bias_w[:], scale=SCALE)
                    if nt < NT - 1:
                        if nt % 2 == 0:
                            nc.scalar.activation(ecat[:, 0, nt, :], e32[:],
                                                 AF.Copy)
                        else:
                            nc.vector.tensor_copy(ecat[:, 0, nt, :], e32[:])
                        nc.gpsimd.tensor_tensor(out=ecat[:, 1, nt, :],
                                                in0=e32[:],
                                                in1=ecat[:, 0, nt, :],
                                                op=OP.subtract)
                        if nt == 0:
                            nc.vector.tensor_copy(macc[:], e32[:])
                        else:
                            nc.vector.tensor_tensor(
                                out=macc[:], in0=e32[:], in1=macc[:],
                                op=OP.max)
                    else:
                        # macc first: it gates the stats transposes and must
                        # not queue behind anything else in the DVE FIFO
                        nc.vector.tensor_tensor(
                            out=macc[:], in0=e32[:], in1=macc[:], op=OP.max)
                        for h in range(HT):
                            hs = slice(h * 256, (h + 1) * 256)
                            nc.scalar.activation(ecat[:, 0, nt, hs], e32[:, hs],
                                                 AF.Copy)
                        # dummy sigmoid: preloads the ACT sigmoid table set
                        # at the earliest possible queue slot, so the scan's
                        # first real sigmoid doesn't pay the 1.28us load
                        dummy = nc.scalar.activation(sigdump[:], bias_w[:],
                                                     AF.Sigmoid)
                        return e32, dummy
                    return e32, None

                def emit_late_er8(w, ecat, e32, h):
                    hs = slice(h * 256, (h + 1) * 256)
                    nc.vector.tensor_tensor(out=ecat[:, 1, NT - 1, hs],
                                            in0=e32[:, hs],
                                            in1=ecat[:, 0, NT - 1, hs],
                                            op=OP.subtract)

                def emit_sum_mms(pssum, e32s, nt):
                    # single accumulation group across all nts AND columns:
                    # a per-column start would re-mark the whole 2KB zero
                    # region and zero the sibling columns' partial sums
                    for c in range(MT):
                        nc.tensor.matmul(
                            pssum[:, c:c + 1],
                            lhsT=e32s[nt][:, c * 128:(c + 1) * 128],
                            rhs=ones32[:], start=(nt == 0 and c == 0),
                            stop=(nt == NT - 1 and c == MT - 1),
                            skip_group_check=True)

                def emit_mm1(w, mid_cb=None):
                    m0 = w * PW
                    ecat = e_pool.tile([128, 2, NT, PW], F8, name=f"ecat{w}",
                                       tag="ecat")
                    macc = acc_pool.tile([128, PW], F32, name=f"macc{w}",
                                         tag="macc")
                    pssum = pss_pool.tile([128, MT], F32, name=f"pss{w}",
                                          tag="pss")
                    e32s = {}
                    for nt in range(NT):
                        ps = ps1_pool.tile([128, PW], F32, name=f"s{w}_{nt}",
                                           tag="ps1")
                        mm1_tile(ps, nt, m0)
                        # row sums: tiny f32 ones-matmuls over the e32 tiles,
                        # two tiles behind so the PE never waits on ACT
                        if nt >= 2:
                            emit_sum_mms(pssum, e32s, nt - 2)
                        e32s[nt], dummy = mm1_evac(w, ecat, macc, ps, nt)
                        if nt == NT // 2 - 1 and mid_cb is not None:
                            mid_cb()
                    return ecat, macc, pssum, e32s, dummy

                def emit_transposes(w, macc):
                    # all 4 transposes pipeline through one psum bank: each
                    # writes its own quarter (single-instruction groups), so
                    # no transpose waits on the previous quarter's reduce
                    mx = stat_pool.tile([128, MT], F32, name=f"mx{w}", tag="mx")
                    pt = pst_pool.tile([128, MT, 128], F32, name=f"tm{w}",
                                       tag="pst")
                    for c in range(MT):
                        nc.tensor.transpose(
                            pt[:, c, :], macc[:, c * 128:(c + 1) * 128],
                            ident[:])
                    for c in range(MT):
                        nc.vector.tensor_reduce(
                            mx[:, c:c + 1], pt[:, c, :], axis=AX.X, op=OP.max)
                    return mx

                def emit_stats_d(w, st, mx, sm):
                    # mx is max_n e_scaled, so em = 4*exp(-scale*row_max) is
                    # exactly its reciprocal -- computed on DVE, keeping the
                    # ACT queue free for the sigmoid table prefetch below
                    nc.vector.reciprocal(st["em"][:], mx[:])
                    tmp = stat_pool.tile([128, MT], F32, name=f"dt{w}",
                                         tag="dtmp")
                    nc.vector.tensor_tensor(out=tmp[:], in0=sm[:],
                                            in1=st["em"][:], op=OP.mult)
                    nc.vector.tensor_scalar_add(st["d"][:], tmp[:], 1.0)

                def emit_mm2_group(w, ecat, mc, dc, otiles, unscaled,
                                   part=None, ps=None, copy_after=None):
                    """One [128m, 512d] psum group, 3-term fp8 DoubleRow.
                    nt14/15-dependent passes are emitted last so the group
                    can start while mm1's tail e-split chain drains.  part
                    'A' emits only h0's nt<14 passes (nothing from the tail
                    e-split); part 'B' emits the rest + the evac copy."""
                    m0 = w * PW
                    if ps is None:
                        ps = ps2_pool.tile([128, 512], F32,
                                           name=f"o{w}_{mc}_{dc}", tag="ps2")
                    ms = slice(mc * 128, (mc + 1) * 128)
                    for h in range(2):
                        if part == "A" and h == 1:
                            break
                        hs = slice(h * 256, (h + 1) * 256)
                        ds = slice(dc * 512 + h * 256, dc * 512 + (h + 1) * 256)
                        if not (part == "B" and h == 0):
                            for nt in range(NT - 2):
                                nc.tensor.matmul(
                                    ps[:, hs], lhsT=ecat[:, :, nt, ms],
                                    rhs=vcat[:, :, nt, ds],
                                    start=(nt == 0), stop=False, perf_mode=PM)
                            for t in range(NT // 2 - 1):
                                nc.tensor.matmul(
                                    ps[:, hs],
                                    lhsT=ecat[:, 0, 2 * t:2 * t + 2, ms],
                                    rhs=vcat[:, 1, 2 * t:2 * t + 2, ds],
                                    start=False, stop=False, perf_mode=PM)
                        if part == "A":
                            return ps
                        t = NT // 2 - 1
                        nc.tensor.matmul(
                            ps[:, hs],
                            lhsT=ecat[:, 0, 2 * t:2 * t + 2, ms],
                            rhs=vcat[:, 1, 2 * t:2 * t + 2, ds],
                            start=False, stop=False, perf_mode=PM)
                        for nt in (NT - 2, NT - 1):
                            nc.tensor.matmul(
                                ps[:, hs], lhsT=ecat[:, :, nt, ms],
                                rhs=vcat[:, :, nt, ds],
                                start=False, stop=(nt == NT - 1), perf_mode=PM)
                    if mc not in otiles:
                        otiles[mc] = out_pool.tile([128, D], BF16,
                                                   name=f"ot{w}_{mc}", tag="ot")
                    oth = otiles[mc][:, dc * 512:(dc + 1) * 512]
                    cp = nc.scalar.activation(oth, ps[:], AF.Copy)
                    if copy_after is not None:
                        add_dep_helper(copy_after.ins, cp.ins, True,
                                       "evac copy trails sigmoid-table dummy")
                    unscaled.append((mc, dc))

                def emit_scan_steps(w, st, sstate, steps):
                    """Sigmoid long-division steps on d [128, 4].  The q
                    update of step i-1 is emitted after step i's z so the
                    DVE executes it inside step i's sigmoid round-trip
                    (strict FIFO); sg alternates so it survives a step."""
                    d_t = st["d"]
                    r0, r1, q0, q1, z, sg0, sg1, t = sstate["tiles"]
                    for i in steps:
                        r = sstate["r"]
                        rn = r1 if r is r0 else r0
                        sg = sg0 if i % 2 == 0 else sg1
                        nc.vector.scalar_tensor_tensor(      # z = 2r - d
                            out=z[:], in0=r[:], scalar=2.0, in1=d_t[:],
                            op0=OP.mult, op1=OP.subtract)
                        if sstate["pending_q"] is not None:
                            pi, psg = sstate["pending_q"]
                            qa = sstate["q"]
                            qn = q1 if qa is q0 else q0
                            nc.vector.scalar_tensor_tensor(  # q' = w*step + q
                                out=qn[:], in0=psg[:],
                                scalar=float(2.0 ** -(pi + 1)),
                                in1=qa[:], op0=OP.mult, op1=OP.add)
                            sstate["q"] = qn
                        nc.scalar.activation(                # step = sig(100 z)
                            sg[:], z[:], AF.Sigmoid, scale=SIG_SCALE)
                        nc.vector.tensor_tensor(             # t = d*step
                            out=t[:], in0=d_t[:], in1=sg[:], op=OP.mult)
                        nc.vector.scalar_tensor_tensor(      # r' = 2r - t
                            out=rn[:], in0=r[:], scalar=2.0, in1=t[:],
                            op0=OP.mult, op1=OP.subtract)
                        sstate["r"] = rn
                        sstate["pending_q"] = (i, sg)
                    if steps and steps[-1] == BITS - 1:
                        pi, psg = sstate["pending_q"]
                        qa = sstate["q"]
                        qn = q1 if qa is q0 else q0
                        nc.vector.scalar_tensor_tensor(
                            out=qn[:], in0=psg[:],
                            scalar=float(2.0 ** -(pi + 1)),
                            in1=qa[:], op0=OP.mult, op1=OP.add)
                        sstate["pending_q"] = None
                        nc.vector.tensor_tensor(
                            out=st["scale"][:], in0=st["em"][:],
                            in1=qn[:], op=OP.mult)

                def make_scan_state(w):
                    tiles = [stat_pool.tile([128, MT], F32, name=f"{nm}{w}",
                                            tag=nm)
                             for nm in ("r0", "r1", "q0", "q1", "z", "sg0",
                                        "sg1", "t")]
                    nc.vector.memset(tiles[0][:], 1.0)
                    nc.vector.memset(tiles[2][:], 0.0)
                    return {"tiles": tiles, "r": tiles[0], "q": tiles[2],
                            "pending_q": None}

                def emit_out(w, st, otiles, unscaled):
                    """Scale the evacuated halves (bf16 in-place, cheap),
                    storing each merged [128, 1024] row block with a single
                    DMA (fewer descriptors -> half the HWDGE cost/byte) as
                    soon as both its halves are scaled."""
                    m0 = w * PW
                    for mc in sorted(otiles):
                        for dc in range(2):
                            oth = otiles[mc][:, dc * 512:(dc + 1) * 512]
                            nc.vector.tensor_scalar_mul(
                                oth, oth, st["scale"][:, mc:mc + 1])
                        nc.sync.dma_start(
                            out=out_d[m0 + mc * 128:m0 + (mc + 1) * 128, :],
                            in_=otiles[mc][:])

                pending_out = []

                def flush_out():
                    while pending_out:
                        emit_out(*pending_out.pop(0))

                for w in range(NWIN):
                    st = {
                        "em": stat_pool.tile([128, MT], F32, name=f"em{w}",
                                             tag="em"),
                        "d": stat_pool.tile([128, MT], F32, name=f"d{w}",
                                            tag="d"),
                        "scale": stat_pool.tile([128, MT], F32, name=f"sc{w}",
                                                tag="sc"),
                    }
                    ecat, macc, pssum, e32s, dummy = emit_mm1(
                        w, mid_cb=flush_out)
                    otiles, unscaled = {}, []
                    # boundary weave: keep the PE busy on work that doesn't
                    # need the nt15 e-split while its chain drains, and put
                    # the DVE reduces ahead of the late er8 halves
                    emit_sum_mms(pssum, e32s, NT - 2)
                    ps_g0 = emit_mm2_group(w, ecat, 0, 0, otiles, unscaled,
                                           part="A")
                    emit_late_er8(w, ecat, e32s[NT - 1], 0)
                    emit_late_er8(w, ecat, e32s[NT - 1], 1)
                    mx = emit_transposes(w, macc)
                    emit_sum_mms(pssum, e32s, NT - 1)
                    sm = stat_pool.tile([128, MT], F32, name=f"sm{w}",
                                        tag="sm")
                    nc.vector.tensor_copy(sm[:], pssum[:])
                    emit_stats_d(w, st, mx, sm)
                    sstate = make_scan_state(w)
                    # dc-major group order so mm2(w0) can start before the
                    # V d[512:] half lands.  Interleave scan emission with
                    # the groups: per-engine FIFOs are strict, so each
                    # group's ACT psum-evac copy must sit between sigmoids
                    # matching its psum-ready time (~2.65 sigmoid paces per
                    # group), and the first sigmoids go before g0's copy.
                    groups = [(mc, dc) for dc in range(2) for mc in range(MT)]
                    sched = [(None, 2), (0, 5), (1, 8), (2, 11), (3, 14),
                             (4, BITS), (5, BITS), (6, BITS), (7, BITS)]
                    done = 0
                    for gi, nsig in sched:
                        if gi == 0:
                            emit_mm2_group(w, ecat, *groups[gi], otiles,
                                           unscaled, part="B", ps=ps_g0,
                                           copy_after=dummy)
                        elif gi is not None:
                            emit_mm2_group(w, ecat, *groups[gi], otiles,
                                           unscaled)
                        emit_scan_steps(w, st, sstate,
                                        list(range(done, nsig)))
                        done = nsig
                    pending_out.append((w, st, otiles, unscaled))
                flush_out()

    nc.compile()
    return nc


def _get_nc():
    if "nc" not in _CACHE:
        _CACHE["nc"] = _build()
    return _CACHE["nc"]


def _split8(x):
    h = x.astype(F8NP)
    l = (x - h.astype(np.float32)).astype(F8NP)
    return np.ascontiguousarray(h), np.ascontiguousarray(l)


def prepare_in_maps(query, keys, values):
    query = np.ascontiguousarray(query, dtype=np.float32)
    keys = np.ascontiguousarray(keys, dtype=np.float32)
    values = np.ascontiguousarray(values, dtype=np.float32)
    k8, kr8 = _split8(np.ascontiguousarray(keys.T))
    v8, vr8 = _split8(values)
    in_maps = []
    for i in range(NCORES):
        qT = np.ascontiguousarray(query[i * M:(i + 1) * M].T)
        q8, qr8 = _split8(qT)
        in_maps.append({"q8": q8, "qr8": qr8, "k8": k8, "kr8": kr8,
                        "v8": v8, "vr8": vr8})
    return in_maps


def kernel(query, keys, values):
    from concourse.bass_utils import run_bass_kernel_spmd

    nc = _get_nc()
    in_maps = prepare_in_maps(query, keys, values)
    res = run_bass_kernel_spmd(nc, in_maps, list(range(NCORES)))
    out = np.concatenate([np.asarray(res.results[i]["out"])
                          for i in range(NCORES)], axis=0)
    return out.astype(np.float32)
